# revision 28
# baseline (speedup 1.0000x reference)
"""Multi-head attention Trainium2 Bass kernel (v3 — merged pipeline).

Problem: B=4, T=2048, D=1024, H=16 heads of dim 64 (fp32).
  qkv = x @ Wqkv.T + bqkv ; per-head attention ; out @ Wo.T + bo

Sharding: 8 cores = 4 batches x 2 head-groups of 8 heads.  Each core
computes its batch's attention for its 8 heads plus the out-projection
restricted to its heads' columns (partial sum); the host adds the two
partial outputs per batch (the "all-reduce") and transposes back.

v3 architecture: one merged pipeline.  QKV generation, attention and
out-projection all share the PE instruction stream; the generation /
out-projection matmul chains are interleaved as "filler" work inside the
attention kt loops so the PE never idles while the ACT engine streams
exps (keeps the PE p-state at full clock on hardware):

  - QK-gen(pack 0) runs first; attention(pack 0) starts immediately.
  - V-gen groups interleave into attention(p0,qh0) one per 2 kt
    (V' tile tt is ready just before the PV matmul that consumes it).
  - QK-gen(pack p+1) groups interleave into attention(pack p).
  - oproj for the qh0 column half interleaves into attention(p3,qh1);
    only the qh1 half of oproj remains as tail.
  - All generation/oproj chains write through the same 2-slot
    [128,1024] PSUM pool the S-matmuls use, so PSUM fits:
    spsum 4 banks + outpsum (PV accumulators) 4 banks = 8.

dtypes: V' (vps, with folded v-bias), P tiles, attnT, Wo, K packs are
bf16 (halves SBUF + PE weight-load time); Q packs / x / Wqk stay f32r.
"""

import sys

sys.path.insert(0, "/opt/trn_rl_repo")

import numpy as np

import concourse.bass as bass  # noqa: F401  (import keeps bass registered)
from concourse import bacc
import concourse.mybir as mybir
import concourse.tile as tile
from concourse.bass_utils import run_bass_kernel_spmd

B, T, D = 4, 2048, 1024
H, HD = 16, 64
P = 128
FP32 = mybir.dt.float32
FP32R = mybir.dt.float32r
BF16 = mybir.dt.bfloat16
AF = mybir.ActivationFunctionType
OP = mybir.AluOpType

N_CORES = 8
HPC = 8          # heads per core
NPACK = 4        # head pairs per core
CT = D // P      # 8 contraction tiles over D
KT = T // P      # 16 key tiles
QH = 2           # q halves of 1024
QHW = T // QH    # 1024
SCALE = HD ** -0.5

BF16_NP = mybir.dt.np(BF16)


def build_nc(reps: int = 1, variant: str = "base", dyn: bool = False):
    nc = bacc.Bacc(None, target_bir_lowering=False, debug=False)

    xT_d = nc.dram_tensor("xT", [D, T], FP32R, kind="ExternalInput")
    wqkT_d = nc.dram_tensor("wqkT", [D, NPACK, 256], FP32R, kind="ExternalInput")
    bqk_d = nc.dram_tensor("bqk", [P, NPACK, 2], FP32, kind="ExternalInput")
    wvT_d = nc.dram_tensor("wvT", [D, HPC * HD], FP32R, kind="ExternalInput")
    bvb_d = nc.dram_tensor("bvb", [P, HPC * HD], FP32, kind="ExternalInput")
    woT_d = nc.dram_tensor("woT", [NPACK * P, D], BF16, kind="ExternalInput")
    bo_d = nc.dram_tensor("bo", [P, CT], FP32, kind="ExternalInput")
    if dyn:
        nreps_d = nc.dram_tensor("nreps", [1, 1], mybir.dt.int32,
                                 kind="ExternalInput")
    yT_d = nc.dram_tensor("yT", [D, T], FP32, kind="ExternalOutput")

    with tile.TileContext(nc) as tc:
        with (
            tc.tile_pool(name="persist", bufs=1) as persist,
        ):
            ones_col = nc.const_aps.tensor(1.0, [P, 1], FP32)

            # ---- persistent SBUF residents --------------------------------
            vps = persist.tile([P, KT, HPC * (HD + 1)], BF16, tag="vps")
            attnT = persist.tile([P, NPACK, T], BF16, tag="attnT")
            qts = persist.tile([P, NPACK, T], BF16, tag="qts")
            kts = persist.tile([P, NPACK, T], BF16, tag="kts")
            woTs = persist.tile([P, NPACK, D], BF16, tag="woTs")
            bqks = persist.tile([P, NPACK, 2], FP32, tag="bqks")
            bvbs_f = persist.tile([P, HPC * HD], FP32, tag="bvbs")
            bvbs = bvbs_f.rearrange("p (h d) -> p h d", h=HPC)
            bos = persist.tile([P, CT], FP32, tag="bos")

            nc.sync.dma_start(bqks[:], bqk_d[:, :, :])
            nc.sync.dma_start(bvbs_f[:], bvb_d[:, :])
            nc.sync.dma_start(bos[:], bo_d[:, :])
            nc.sync.dma_start(
                woTs[:], woT_d.rearrange("(c p) d -> p c d", p=P))

            if dyn:
                nrt_sb = persist.tile([1, 1], mybir.dt.int32, tag="nrt")
                nc.sync.dma_start(nrt_sb[:], nreps_d[:, :])
                nval = nc.values_load(nrt_sb[0:1, 0:1], min_val=1,
                                      max_val=1 << 20,
                                      skip_runtime_bounds_check=True)
                rep_ctx = tc.For_i(0, nval, 1)
            else:
                rep_ctx = None

            with tc.tile_pool(name="xts_pool", bufs=1) as xts_pool:
                xts = xts_pool.tile([P, CT, T], FP32R, tag="xts")

                import contextlib
                with rep_ctx if rep_ctx is not None else contextlib.nullcontext():
                  for _ in range(1 if dyn else reps):
                    with (
                        tc.tile_pool(name="wqk_pool", bufs=2) as wqk_pool,
                        tc.tile_pool(name="wv_pool", bufs=1) as wv_pool,
                        tc.tile_pool(name="pt_pool", bufs=5) as pt_pool,
                        tc.tile_pool(name="rep_pool", bufs=2) as rep_pool,
                        tc.tile_pool(name="ystage_pool", bufs=2) as ystage,
                        tc.tile_pool(name="spsum", bufs=2,
                                     space="PSUM") as spsum,
                        tc.tile_pool(name="outpsum", bufs=2,
                                     space="PSUM") as outpsum,
                    ):
                        # ---- DMAs: pack-0 weights first, then x (chunked
                        # token-major so gen chases the DMA stream), V/W ----
                        wqks = [None] * NPACK
                        wqks[0] = wqk_pool.tile([P, CT, 256], FP32R, tag="wqk",
                                                name="wqk0")
                        nc.sync.dma_start(
                            wqks[0][:],
                            wqkT_d[:, 0, :].rearrange("(c p) w -> p c w", p=P))
                        wvs = wv_pool.tile([P, CT, HPC * HD], FP32R, tag="wvs")
                        nc.sync.dma_start(
                            wvs[:], wvT_d.rearrange("(c p) w -> p c w", p=P))
                        for tb in range(T // 512):
                            nc.sync.dma_start(
                                xts[:, :, tb * 512:(tb + 1) * 512],
                                xT_d[:, tb * 512:(tb + 1) * 512]
                                .rearrange("(c p) t -> p c t", p=P))

                        # ones columns of V' (col 64 of each 65-col block)
                        for tt in range(KT):
                            vview = vps[:, tt, :].rearrange(
                                "p (h e) -> p h e", h=HPC)
                            nc.vector.tensor_copy(
                                vview[:, :, HD:HD + 1],
                                ones_col.to_broadcast([P, HPC, 1]))

                        # ---- filler half-group emitters (each ~8 matmuls
                        # into half of one shared-pool PSUM slot) ---------
                        def load_wqk(p):
                            wqks[p] = wqk_pool.tile([P, CT, 256], FP32R,
                                                    tag="wqk", name=f"wqk{p}")
                            nc.sync.dma_start(
                                wqks[p][:],
                                wqkT_d[:, p, :].rearrange(
                                    "(c p) w -> p c w", p=P))

                        def emit_qk_half(p, jj, tb):
                            # 512 tokens of Q (jj=0) or K (jj=1) for pack p
                            ps = spsum.tile([P, QHW], FP32, tag="sps",
                                            name="psg")
                            for ct in range(CT):
                                nc.tensor.matmul(
                                    ps[:, 0:512],
                                    wqks[p][:, ct, jj * P:(jj + 1) * P],
                                    xts[:, ct, tb * 512:(tb + 1) * 512],
                                    start=(ct == 0), stop=(ct == CT - 1))
                            dst = qts if jj == 0 else kts
                            nc.vector.tensor_scalar_add(
                                dst[:, p, tb * 512:(tb + 1) * 512],
                                ps[:, 0:512], bqks[:, p, jj:jj + 1])

                        def emit_v_half(tt):
                            # token tile tt of V' = V + bv
                            ps = spsum.tile([P, QHW], FP32, tag="sps",
                                            name="psg")
                            for ct in range(CT):
                                nc.tensor.matmul(
                                    ps[:, 0:HPC * HD],
                                    xts[:, ct, tt * P:(tt + 1) * P],
                                    wvs[:, ct, :],
                                    start=(ct == 0), stop=(ct == CT - 1))
                            vview = vps[:, tt, :].rearrange(
                                "p (h e) -> p h e", h=HPC)
                            nc.vector.tensor_tensor(
                                vview[:, :, 0:HD],
                                ps[:, 0:HPC * HD]
                                .rearrange("p (h d) -> p h d", h=HPC),
                                bvbs[:], OP.add)

                        def emit_oproj_half(co, tb):
                            # output dims [co*128, +128) for 512 tokens
                            ps = spsum.tile([P, QHW], FP32, tag="sps",
                                            name="psg")
                            for ci in range(NPACK):
                                nc.tensor.matmul(
                                    ps[:, 0:512],
                                    woTs[:, ci, co * P:(co + 1) * P],
                                    attnT[:, ci, tb * 512:(tb + 1) * 512],
                                    start=(ci == 0), stop=(ci == NPACK - 1))
                            yst = ystage.tile([P, 512], FP32, tag="yst")
                            nc.vector.tensor_scalar_add(
                                yst[:], ps[:, 0:512], bos[:, co:co + 1])
                            nc.sync.dma_start(
                                yT_d[co * P:(co + 1) * P,
                                     tb * 512:(tb + 1) * 512], yst[:])

                        # ---- global filler plan: (window, kt) -> thunks --
                        # window = p * QH + qh; 16 kts per window.
                        plan = {}

                        def sched(w, kt, thunk):
                            plan.setdefault((w, kt), []).append(thunk)

                        # V half tt inside window 0 at kt=tt (ready one kt
                        # before PV needs vps[tt])
                        for tt in range(KT):
                            sched(0, tt, lambda tt=tt: emit_v_half(tt))
                        sched(0, 1, lambda: load_wqk(1))
                        # QK pack 1: 8 halves over window 1, every 2nd kt
                        for i in range(8):
                            jj, tb = i // 4, i % 4
                            sched(1, 2 * i, lambda jj=jj, tb=tb:
                                  emit_qk_half(1, jj, tb))
                        sched(1, 1, lambda: load_wqk(2))
                        # QK pack 2 over windows 2-3; pack 3 over 4-5
                        for i in range(8):
                            jj, tb = i // 4, i % 4
                            sched(2 + i // 4, 4 * (i % 4) + 1,
                                  lambda jj=jj, tb=tb: emit_qk_half(2, jj, tb))
                        sched(3, 2, lambda: load_wqk(3))
                        for i in range(8):
                            jj, tb = i // 4, i % 4
                            sched(4 + i // 4, 4 * (i % 4) + 2,
                                  lambda jj=jj, tb=tb: emit_qk_half(3, jj, tb))
                        # oproj qh0 half (tb 0,1): window 7, after normalize
                        # of (p3,qh0) which is emitted at (7, kt=0)
                        for i in range(KT):
                            co, tb = i // 2, i % 2
                            sched(7, 2 + (i * 14) // KT,
                                  lambda co=co, tb=tb: emit_oproj_half(co, tb))

                        if variant == "attonly":
                            # skip generation: zero qts/kts/vps, no fillers
                            nc.vector.memset(
                                qts[:].bitcast(mybir.dt.uint16), 0)
                            nc.vector.memset(
                                kts[:].bitcast(mybir.dt.uint16), 0)
                            nc.vector.memset(
                                vps[:].bitcast(mybir.dt.uint16), 0)
                            plan = {}
                        elif variant == "genonly":
                            # generation + oproj only: no S/exp/PV/normalize
                            nc.vector.memset(
                                attnT[:].bitcast(mybir.dt.uint16), 0)
                            for jj in range(2):
                                for tb in range(4):
                                    emit_qk_half(0, jj, tb)
                            for (w, kt) in sorted(plan):
                                for thunk in plan[(w, kt)]:
                                    thunk()
                            for co in range(CT):
                                for tb in (2, 3):
                                    emit_oproj_half(co, tb)

                        # ---- QK gen for pack 0 (lead-in) ----------------
                        if variant not in ("attonly", "genonly"):
                            for jj in range(2):
                                for tb in range(4):
                                    emit_qk_half(0, jj, tb)

                        # ---- attention: one flattened software-pipelined
                        # loop over all (pack, qh) windows ----------------
                        def emit_pv(entry):
                            # sh-outer so consecutive matmuls alternate
                            # stationaries (same-weights back-to-back defeats
                            # the PE weight double-buffer: ~4x slower)
                            ktp, pts, halves_w = entry
                            for sh in range(QHW // 512):
                                for (lo, outp, hloc), pt in zip(halves_w, pts):
                                    nc.tensor.matmul(
                                        outp[0:HD + 1,
                                             sh * 512:(sh + 1) * 512],
                                        vps[:, ktp,
                                            hloc * (HD + 1):
                                            (hloc + 1) * (HD + 1)],
                                        pt[:, sh * 512:(sh + 1) * 512],
                                        start=(ktp == 0),
                                        stop=(ktp == KT - 1))

                        def emit_norm(entry_halves, p, qh):
                            q0 = qh * QHW
                            for (lo, outp, hloc) in entry_halves:
                                row0 = lo
                                rep = rep_pool.tile([HD, QHW], FP32, tag="rep")
                                nc.vector.reciprocal(
                                    rep[0:1, :], outp[HD:HD + 1, :])
                                nc.gpsimd.partition_broadcast(
                                    rep[:], rep[0:1, :])
                                dst = attnT[row0:row0 + HD, p, q0:q0 + QHW]
                                nc.vector.tensor_tensor(
                                    dst, outp[0:HD, :], rep[:], OP.mult)

                        prev = None          # (ktp, pts, halves) pending PV
                        done_win = None      # (halves, p, qh) pending norm
                        for w in range(0 if variant != "genonly"
                                       else NPACK * QH, NPACK * QH):
                            p, qh = w // QH, w % QH
                            hA, hB = 2 * p, 2 * p + 1
                            q0 = qh * QHW
                            outA = outpsum.tile([P, QHW], FP32, tag="outp",
                                                name="outA")
                            outB = outpsum.tile([P, QHW], FP32, tag="outp",
                                                name="outB")
                            halves = [(0, outA, hA), (HD, outB, hB)]
                            for kt in range(KT):
                                sls = [spsum.tile([P, QHW], FP32, tag="sps",
                                                  name=f"sps{h}")
                                       for h in range(2)]
                                # interleave A/B so disjoint row-group
                                # matmuls overlap in the PE array
                                nspass = 2 if variant == "dblmm" else 1
                                for _sp in range(nspass):
                                    for sh in range(QHW // 512):
                                        for (lo, outp, hloc), sps in zip(
                                                halves, sls):
                                            nc.tensor.matmul(
                                                sps[:, sh * 512:(sh + 1) * 512],
                                                kts[lo:lo + HD, p,
                                                    kt * P:(kt + 1) * P],
                                                qts[lo:lo + HD, p,
                                                    q0 + sh * 512:
                                                    q0 + (sh + 1) * 512],
                                                start=True, stop=True)
                                pts = []
                                for sps in sls:
                                    pt = pt_pool.tile([P, QHW], BF16, tag="pt")
                                    nc.scalar.activation(
                                        pt[:], sps[:], AF.Exp, scale=SCALE)
                                    pts.append(pt)
                                if prev is not None:
                                    emit_pv(prev)
                                prev = (kt, pts, halves)
                                if done_win is not None:
                                    emit_norm(*done_win)
                                    done_win = None
                                thunks = plan.get((w, kt), ())
                                for thunk in thunks:
                                    thunk()
                                if variant == "pad" and not thunks:
                                    # redundant rewrite of the sh0 S-scores
                                    # (same values) — bridges the PE idle so
                                    # the clock p-state stays at max
                                    for (lo, outp, hloc), sps in zip(
                                            halves, sls):
                                        nc.tensor.matmul(
                                            sps[:, 0:512],
                                            kts[lo:lo + HD, p,
                                                kt * P:(kt + 1) * P],
                                            qts[lo:lo + HD, p,
                                                q0:q0 + 512],
                                            start=True, stop=True)
                            done_win = (halves, p, qh)
                        if variant != "genonly":
                            emit_pv(prev)
                            emit_norm(*done_win)
                            # ---- out projection tail (qh1 half) ---------
                            for co in range(CT):
                                for tb in (2, 3):
                                    emit_oproj_half(co, tb)
    nc.compile()
    return nc


def _prep_core_inputs(x, Wqkv, bqkv, Wo, bo, core):
    b, g = core // 2, core % 2
    f32 = np.float32

    xT = np.ascontiguousarray(x[b].T, dtype=f32)

    wqkT = np.empty((D, NPACK, 256), f32)
    bqk = np.empty((P, NPACK, 2), f32)
    for p in range(NPACK):
        rows_q, rows_k = [], []
        for j in range(2):
            h = 8 * g + 2 * p + j
            rows_q.append(slice(192 * h, 192 * h + 64))
            rows_k.append(slice(192 * h + 64, 192 * h + 128))
        Q2 = np.vstack([Wqkv[rows_q[0]], Wqkv[rows_q[1]]])   # [128, D]
        K2 = np.vstack([Wqkv[rows_k[0]], Wqkv[rows_k[1]]])
        wqkT[:, p, :128] = Q2.T
        wqkT[:, p, 128:] = K2.T
        bqk[:, p, 0] = np.concatenate([bqkv[rows_q[0]], bqkv[rows_q[1]]])
        bqk[:, p, 1] = np.concatenate([bqkv[rows_k[0]], bqkv[rows_k[1]]])

    rows_v = [slice(192 * (8 * g + h) + 128, 192 * (8 * g + h) + 192)
              for h in range(HPC)]
    Wv = np.vstack([Wqkv[r] for r in rows_v])                # [512, D]
    wvT = np.ascontiguousarray(Wv.T, dtype=f32)
    bv_flat = np.concatenate([np.asarray(bqkv[r], f32) for r in rows_v])
    bvb = np.broadcast_to(bv_flat[None, :], (P, HPC * HD)).copy()

    woT = np.ascontiguousarray(Wo[:, 512 * g:512 * (g + 1)].T).astype(BF16_NP)
    bo2 = (bo.reshape(CT, P).T.astype(f32).copy() if g == 0
           else np.zeros((P, CT), f32))

    return {
        "xT": xT, "wqkT": wqkT, "bqk": bqk, "wvT": wvT,
        "bvb": bvb, "woT": woT, "bo": bo2,
    }


_NC_CACHE = {}


def kernel(x, Wqkv, bqkv, Wo, bo, _reps: int = 1,
           _return_raw: bool = False):
    x = np.asarray(x, np.float32)
    Wqkv = np.asarray(Wqkv, np.float32)
    bqkv = np.asarray(bqkv, np.float32)
    Wo = np.asarray(Wo, np.float32)
    bo = np.asarray(bo, np.float32)

    in_maps = [_prep_core_inputs(x, Wqkv, bqkv, Wo, bo, c)
               for c in range(N_CORES)]

    if _reps not in _NC_CACHE:
        _NC_CACHE[_reps] = build_nc(_reps)
    nc = _NC_CACHE[_reps]

    res = run_bass_kernel_spmd(nc, in_maps, core_ids=list(range(N_CORES)))
    if _return_raw:
        return res

    y = np.empty((B, T, D), np.float32)
    for b in range(B):
        yt = res.results[2 * b]["yT"] + res.results[2 * b + 1]["yT"]
        y[b] = yt.T
    return y
